# revision 16
# baseline (speedup 1.0000x reference)
"""Trainium2 Bass kernel for nn_MultiHeadAttention_31765578121852.

Sharding: 8 cores = 2 batches x 4 head-groups (4 heads each).
Per core: QKV projection (RoPE folded into weights on host), causal
flash-attention, partial output projection (Wout rows for this head
group); host sums partials over head groups.

Returns (out, k, v) matching the reference.
"""
import sys
for p in ("/opt/trn_rl_repo", "/root/.axon_site/_ro/trn_rl_repo"):
    if p not in sys.path:
        sys.path.insert(0, p)

import math
import numpy as np

import concourse.bacc as bacc
import concourse.mybir as mybir
from concourse.tile import TileContext
from concourse.bass_utils import run_bass_kernel_spmd

# Problem constants (hardcoded per contract)
B, T, D = 2, 2048, 2048
N_HEAD, HEAD_DIM = 16, 128
HG = 4                      # heads per core
N_CORES = 8
P = 128                     # partitions
NT = T // P                 # 16 q-tiles
NCH = T // 512              # 4 max score chunks

F32 = mybir.dt.float32
F32R = mybir.dt.float32r
F16 = mybir.dt.float16

_NC_CACHE = {}


def build_nc(profile_label=""):
    if "nc" in _NC_CACHE:
        return _NC_CACHE["nc"]
    nc = bacc.Bacc("TRN2")

    # ---- DRAM I/O (per-core shapes) ----
    xth = nc.dram_tensor("xth", [D, T], F32R, kind="ExternalInput")       # x[b].T hi
    xtl = nc.dram_tensor("xtl", [D, T], F32R, kind="ExternalInput")       # x[b].T lo
    wqkh = nc.dram_tensor("wqkh", [D, 2 * HG * P], F32R, kind="ExternalInput")
    wqkl = nc.dram_tensor("wqkl", [D, 2 * HG * P], F32R, kind="ExternalInput")
    wvh = nc.dram_tensor("wvh", [D, HG * P], F32R, kind="ExternalInput")
    wvl = nc.dram_tensor("wvl", [D, HG * P], F32R, kind="ExternalInput")
    wo = nc.dram_tensor("wo", [HG * P, D], F32, kind="ExternalInput")
    t128 = nc.dram_tensor("t128", [P, T], F32, kind="ExternalInput")
    qscale = nc.dram_tensor("qscale", [P, NT], F32, kind="ExternalInput")
    trimask = nc.dram_tensor("trimask", [P, P], F32, kind="ExternalInput")

    outp = nc.dram_tensor("outp", [T, D], F32, kind="ExternalOutput")
    khat = nc.dram_tensor("khat", [HG, P, T], F32, kind="ExternalOutput")
    vout = nc.dram_tensor("vout", [T, HG * P], F32, kind="ExternalOutput")

    with TileContext(nc) as tc:
        with (
            tc.tile_pool(name="persist", bufs=1) as pp,
            tc.tile_pool(name="const", bufs=1) as cp,
        ):
            # Persistent SBUF tensors
            qt_all = pp.tile([P, HG * T], F32, tag="qt")       # Q-tilde, [d, T] per head
            kh_all = pp.tile([P, HG * T], F32, tag="kh")       # K-hat,  [d, T] per head
            vr_all = pp.tile([P, NT * HG * P], F16, tag="vr")  # V fp16, [T-tile, hg*128]
            ot_all = pp.tile([P, HG * T], F32R, tag="ot")      # O^T fp32r per head

            sc_q = cp.tile([P, NT], F32, tag="scq")
            mask_t = cp.tile([P, P], F32, tag="mask")
            nc.sync.dma_start(sc_q[:], qscale[:])
            nc.sync.dma_start(mask_t[:], trimask[:])

            # ================= Phase 1a: Q/K projection (split fp32r) =====
            with (
                tc.tile_pool(name="p1", bufs=4) as sp1,
                tc.tile_pool(name="t128p", bufs=1) as tp,
                tc.tile_pool(name="ps1", bufs=8, space="PSUM") as ps1,
            ):
                t128_t = tp.tile([P, T], F32, tag="t128")
                nc.sync.dma_start(t128_t[:], t128[:])
                for n in range(4):          # T chunks of 512
                    nsl = slice(n * 512, (n + 1) * 512)
                    psm = [ps1.tile([P, 512], F32, tag="qk_ps", name=f"qkps_{n}_{m}") for m in range(8)]
                    for kc in range(16):
                        xth_t = sp1.tile([P, 512], F32R, tag="xth1")
                        xtl_t = sp1.tile([P, 512], F32R, tag="xtl1")
                        nc.sync.dma_start(xth_t[:], xth[kc * P:(kc + 1) * P, nsl])
                        nc.sync.dma_start(xtl_t[:], xtl[kc * P:(kc + 1) * P, nsl])
                        wqkh_t = sp1.tile([P, 1024], F32R, tag="wqkh")
                        wqkl_t = sp1.tile([P, 1024], F32R, tag="wqkl")
                        nc.sync.dma_start(wqkh_t[:], wqkh[kc * P:(kc + 1) * P, :])
                        nc.sync.dma_start(wqkl_t[:], wqkl[kc * P:(kc + 1) * P, :])
                        for m in range(8):
                            msl = slice(m * P, (m + 1) * P)
                            nc.tensor.matmul(psm[m][:], wqkh_t[:, msl], xth_t[:],
                                             start=(kc == 0), stop=False)
                            nc.tensor.matmul(psm[m][:], wqkl_t[:, msl], xth_t[:],
                                             start=False, stop=False)
                            nc.tensor.matmul(psm[m][:], wqkh_t[:, msl], xtl_t[:],
                                             start=False, stop=(kc == 15))
                    for m in range(8):
                        if m < 4:
                            nc.scalar.copy(qt_all[:, m * T + n * 512: m * T + (n + 1) * 512], psm[m][:])
                        else:
                            h = m - 4
                            dst = kh_all[:, h * T + n * 512: h * T + (n + 1) * 512]
                            nc.vector.tensor_mul(dst, psm[m][:], t128_t[:, nsl])

            # ================= Phase 1b: V projection (split fp32r) =======
            with (
                tc.tile_pool(name="p1b", bufs=4) as sp2,
                tc.tile_pool(name="ps2", bufs=6, space="PSUM") as ps2,
            ):
                for n in range(4):
                    nc.sync.dma_start(khat[n], kh_all[:, n * T:(n + 1) * T])
                    psv = [ps2.tile([P, 512], F32, tag="v_ps", name=f"vps_{n}_{j}") for j in range(4)]
                    for kc in range(16):
                        xth_t = sp2.tile([P, 512], F32R, tag="xth2")
                        xtl_t = sp2.tile([P, 512], F32R, tag="xtl2")
                        nc.sync.dma_start(xth_t[:], xth[kc * P:(kc + 1) * P, n * 512:(n + 1) * 512])
                        nc.sync.dma_start(xtl_t[:], xtl[kc * P:(kc + 1) * P, n * 512:(n + 1) * 512])
                        wvh_t = sp2.tile([P, 512], F32R, tag="wvh")
                        wvl_t = sp2.tile([P, 512], F32R, tag="wvl")
                        nc.sync.dma_start(wvh_t[:], wvh[kc * P:(kc + 1) * P, :])
                        nc.sync.dma_start(wvl_t[:], wvl[kc * P:(kc + 1) * P, :])
                        for j in range(4):
                            jsl = slice(j * P, (j + 1) * P)
                            nc.tensor.matmul(psv[j][:], xth_t[:, jsl], wvh_t[:],
                                             start=(kc == 0), stop=False)
                            nc.tensor.matmul(psv[j][:], xtl_t[:, jsl], wvh_t[:],
                                             start=False, stop=False)
                            nc.tensor.matmul(psv[j][:], xth_t[:, jsl], wvl_t[:],
                                             start=False, stop=(kc == 15))
                    for j in range(4):
                        tt = n * 4 + j
                        vst = sp2.tile([P, 512], F32, tag="vstage")
                        nc.scalar.copy(vst[:], psv[j][:])
                        nc.sync.dma_start(vout[tt * P:(tt + 1) * P, :], vst[:])
                        nc.vector.tensor_copy(vr_all[:, tt * 512:(tt + 1) * 512], vst[:])

            wo_ts = []
            for h in range(HG):
                wo_t = pp.tile([P, D], F32R, tag=f"wo{h}", name=f"wo_{h}")
                nc.gpsimd.dma_start(wo_t[:], wo[h * P:(h + 1) * P, :])
                wo_ts.append(wo_t)

            # ========== Phase 2: attention + fused output projection ======
            with (
                tc.tile_pool(name="p2", bufs=3) as ap,
                tc.tile_pool(name="p2s", bufs=4) as aps,
                tc.tile_pool(name="ptp", bufs=2) as ptp,
                tc.tile_pool(name="ps_s", bufs=6, space="PSUM") as psA,
                tc.tile_pool(name="ps_o", bufs=2, space="PSUM") as psO,
            ):
                def softmax_qtile(h, i, ptile):
                    hq = qt_all[:, h * T:(h + 1) * T]
                    hk = kh_all[:, h * T:(h + 1) * T]
                    W = P * (i + 1)
                    widths = [512] * (W // 512) + ([W % 512] if W % 512 else [])
                    nch = len(widths)
                    mx = aps.tile([P, 4], F32, tag="mx", name=f"mx_{h}_{i}")
                    dn = aps.tile([P, 4], F32, tag="dn", name=f"dn_{h}_{i}")
                    chunks = []
                    c0 = 0
                    for ci, wd in enumerate(widths):
                        s_ps = psA.tile([P, 512], F32, tag="ps", name=f"s_{h}_{i}_{ci}")
                        nc.tensor.matmul(
                            s_ps[:, :wd], hq[:, i * P:(i + 1) * P], hk[:, c0:c0 + wd],
                            start=True, stop=True)
                        if ci == nch - 1:
                            doff = wd - P
                            nc.vector.tensor_add(
                                s_ps[:, doff:doff + P], s_ps[:, doff:doff + P], mask_t[:])
                        nc.vector.reduce_max(
                            mx[:, ci:ci + 1], s_ps[:, :wd], axis=mybir.AxisListType.X)
                        chunks.append((s_ps, c0, wd))
                        c0 += wd
                    m_f = aps.tile([P, 1], F32, tag="mf", name=f"mf_{h}_{i}")
                    bias = aps.tile([P, 1], F32, tag="bias", name=f"bias_{h}_{i}")
                    nc.vector.reduce_max(m_f[:], mx[:, :nch], axis=mybir.AxisListType.X)
                    nc.vector.tensor_scalar(
                        out=bias[:], in0=m_f[:], scalar1=sc_q[:, i:i + 1],
                        scalar2=-1.0, op0=mybir.AluOpType.mult,
                        op1=mybir.AluOpType.mult)
                    for ci, (s_ps, c0, wd) in enumerate(chunks):
                        nc.scalar.activation(
                            ptile[:, c0:c0 + wd], s_ps[:, :wd],
                            mybir.ActivationFunctionType.Exp,
                            bias=bias[:, 0:1], scale=sc_q[:, i:i + 1],
                            accum_out=dn[:, ci:ci + 1])
                    dsum = aps.tile([P, 1], F32, tag="dsum", name=f"ds_{h}_{i}")
                    rec = aps.tile([P, 1], F32, tag="rec", name=f"rec_{h}_{i}")
                    nc.vector.reduce_sum(dsum[:], dn[:, :nch], axis=mybir.AxisListType.X)
                    nc.vector.reciprocal(rec[:], dsum[:])
                    nc.vector.tensor_scalar_mul(ptile[:, :W], ptile[:, :W], rec[:, 0:1])

                for pr in range(NT // 2):     # q-pairs
                    i0, i1 = 2 * pr, 2 * pr + 1

                    def pv_block(h, big_pt):
                        ot_ps = psO.tile([P, 256], F32, tag="ot", name=f"ot_{h}_{pr}")
                        for kt in range(i1 + 1):
                            nc.tensor.matmul(
                                ot_ps[:, 0:256],
                                vr_all[:, kt * 512 + h * P: kt * 512 + (h + 1) * P],
                                big_pt[:, kt, :],
                                start=(kt == 0), stop=(kt == i1))
                        nc.vector.tensor_copy(
                            ot_all[:, h * T + i0 * P: h * T + (i1 + 1) * P], ot_ps[:, 0:256])

                    for h in range(HG):
                        pt0 = ap.tile([P, T], F16, tag="pt0", name=f"p0_{h}_{pr}")
                        pt1 = ap.tile([P, T], F16, tag="pt1", name=f"p1_{h}_{pr}")
                        softmax_qtile(h, i0, pt0)
                        softmax_qtile(h, i1, pt1)
                        big_pt = ptp.tile([P, NT, 256], F16, tag="bigpt", name=f"bpt_{h}_{pr}")
                        nc.sync.dma_start_transpose(
                            big_pt[:, 0:i1, 0:P], pt0[:, 0:i1 * P])
                        nc.gpsimd.memset(big_pt[:, i1, 0:P], 0.0)
                        nc.sync.dma_start_transpose(
                            big_pt[:, 0:i1 + 1, P:256], pt1[:, 0:(i1 + 1) * P])
                        pv_block(h, big_pt)

            # -------- output projection (separate tail phase) --------
            with (
                tc.tile_pool(name="p3", bufs=3) as op,
                tc.tile_pool(name="ps_w", bufs=4, space="PSUM") as psW,
            ):
                for pr in range(NT // 2):
                    for half, tt in enumerate((2 * pr, 2 * pr + 1)):
                        for n in range(4):
                            po = psW.tile([P, 512], F32, tag="po", name=f"po_{pr}_{half}_{n}")
                            for h in range(HG):
                                nc.tensor.matmul(
                                    po[:],
                                    ot_all[:, h * T + tt * P: h * T + (tt + 1) * P],
                                    wo_ts[h][:, n * 512:(n + 1) * 512],
                                    start=(h == 0), stop=(h == HG - 1))
                            ost = op.tile([P, 512], F32, tag="ostage", name=f"os_{pr}_{half}_{n}")
                            nc.scalar.copy(ost[:], po[:])
                            nc.sync.dma_start(outp[tt * P:(tt + 1) * P, n * 512:(n + 1) * 512], ost[:])

    nc.finalize()
    _NC_CACHE["nc"] = nc
    return nc


def _host_prep(x, Wqkv, Wout):
    """Build per-core input maps. RoPE folded into Wq/Wk (exact: cos/sin
    are linear in t for this reference)."""
    D2 = HEAD_DIM // 2
    inv = 1.0 / (10000.0 ** (np.arange(0, HEAD_DIM, 2, dtype=np.float64) / HEAD_DIM))
    emb1 = np.concatenate([inv, inv])           # freqs row at t=1
    alpha = emb1[::2].copy()                    # [64]
    beta = emb1[1::2].copy()                    # [64]

    Wq64 = np.asarray(Wqkv[:, 0:D], np.float64)
    Wk64 = np.asarray(Wqkv[:, D:2 * D], np.float64)

    tvals = np.arange(T, dtype=np.float64)
    t128v = np.broadcast_to(tvals[None, :].astype(np.float32), (P, T)).copy()
    qs = np.empty((P, NT), np.float32)
    for i in range(NT):
        tq = np.maximum(np.arange(i * P, (i + 1) * P, dtype=np.float64), 1e-3)
        qs[:, i] = (tq / math.sqrt(HEAD_DIM)).astype(np.float32)
    tri = np.where(np.arange(P)[:, None] >= np.arange(P)[None, :], 0.0, -1e30).astype(np.float32)

    def split12(a):
        a = np.ascontiguousarray(a, np.float32)
        bits = a.view(np.uint32)
        hi = ((bits + 0x800) & 0xFFFFF000).view(np.float32).copy()
        hi[~np.isfinite(hi)] = a[~np.isfinite(hi)]
        lo = (a - hi).astype(np.float32)
        return hi, lo

    def fold(Wh):  # Wh [D, 128] fp64 -> folded [D, 128] (d-tilde order)
        We, Wo_ = Wh[:, 0::2], Wh[:, 1::2]      # [D, 64] each
        lo = We * alpha[None, :] - Wo_ * beta[None, :]
        hi = We * beta[None, :] + Wo_ * alpha[None, :]
        return np.concatenate([lo, hi], axis=1)

    perm = np.empty(HEAD_DIM, np.int64)
    perm[:D2] = np.arange(0, HEAD_DIM, 2)       # d-tilde j -> original dim
    perm[D2:] = np.arange(1, HEAD_DIM, 2)

    in_maps = []
    for c in range(N_CORES):
        b, hg = divmod(c, HG)
        heads = range(hg * HG, hg * HG + HG)
        wq_f = np.concatenate([fold(Wq64[:, gh * P:(gh + 1) * P]) for gh in heads], axis=1)
        wk_f = np.concatenate([fold(Wk64[:, gh * P:(gh + 1) * P]) for gh in heads], axis=1)
        wqk = np.concatenate([wq_f, wk_f], axis=1).astype(np.float32)
        wqkh, wqkl = split12(wqk)
        wv = np.ascontiguousarray(
            Wqkv[:, 2 * D + hg * HG * P: 2 * D + (hg + 1) * HG * P]).astype(np.float32)
        wvh, wvl = split12(wv)
        wo_rows = np.ascontiguousarray(
            np.asarray(Wout)[hg * HG * P:(hg + 1) * HG * P, :]).astype(np.float32)
        xt = np.ascontiguousarray(np.asarray(x)[b].T).astype(np.float32)
        xth, xtl = split12(xt)
        in_maps.append({
            "xth": xth, "xtl": xtl, "wqkh": wqkh, "wqkl": wqkl,
            "wvh": wvh, "wvl": wvl, "wo": wo_rows,
            "t128": t128v, "qscale": qs, "trimask": tri,
        })
    return in_maps, perm


def kernel(x, attn_mask, Wqkv, Wout, _trace=False):
    x = np.asarray(x); Wqkv = np.asarray(Wqkv); Wout = np.asarray(Wout)
    in_maps, perm = _host_prep(x, Wqkv, Wout)
    nc = build_nc()
    res = run_bass_kernel_spmd(nc, in_maps, core_ids=list(range(N_CORES)),
                               trace=_trace)

    out = np.zeros((B, T, D), np.float32)
    k = np.empty((B, N_HEAD, T, HEAD_DIM), np.float32)
    v = np.empty((B, N_HEAD, T, HEAD_DIM), np.float32)
    for c in range(N_CORES):
        b, hg = divmod(c, HG)
        r = res.results[c]
        out[b] += r["outp"]
        for h in range(HG):
            gh = hg * HG + h
            tmp = np.empty((HEAD_DIM, T), np.float32)
            tmp[perm] = r["khat"][h]
            k[b, gh] = tmp.T
            v[b, gh] = r["vout"][:, h * P:(h + 1) * P]
    if _trace:
        kernel.last_exec_time_ns = res.exec_time_ns
    return (out, k, v)


if __name__ == "__main__":
    rng = np.random.default_rng(0)
    x = rng.standard_normal((B, T, D)).astype(np.float32)
    Wqkv = (rng.standard_normal((D, 3 * D)) / math.sqrt(D)).astype(np.float32)
    Wout = (rng.standard_normal((D, D)) / math.sqrt(D)).astype(np.float32)
    mask = np.tril(np.ones((T, T), np.int32))[None, None]
    o, kk, vv = kernel(x, mask, Wqkv, Wout)
    print("ran ok", o.shape, kk.shape, vv.shape)


# revision 17
# speedup vs baseline: 1.0950x; 1.0950x over previous
"""Trainium2 Bass kernel for nn_MultiHeadAttention_31765578121852.

Sharding: 8 cores = 2 batches x 4 head-groups (4 heads each).
Per core: QKV projection (RoPE folded into weights on host), causal
flash-attention, partial output projection (Wout rows for this head
group); host sums partials over head groups.

Returns (out, k, v) matching the reference.
"""
import sys
for p in ("/opt/trn_rl_repo", "/root/.axon_site/_ro/trn_rl_repo"):
    if p not in sys.path:
        sys.path.insert(0, p)

import math
import numpy as np

import concourse.bacc as bacc
import concourse.mybir as mybir
from concourse.tile import TileContext
from concourse.bass_utils import run_bass_kernel_spmd

# Problem constants (hardcoded per contract)
B, T, D = 2, 2048, 2048
N_HEAD, HEAD_DIM = 16, 128
HG = 4                      # heads per core
N_CORES = 8
P = 128                     # partitions
NT = T // P                 # 16 q-tiles
NCH = T // 512              # 4 max score chunks

F32 = mybir.dt.float32
F32R = mybir.dt.float32r
F16 = mybir.dt.float16

_NC_CACHE = {}


def build_nc(profile_label=""):
    if "nc" in _NC_CACHE:
        return _NC_CACHE["nc"]
    nc = bacc.Bacc("TRN2")

    # ---- DRAM I/O (per-core shapes) ----
    xth = nc.dram_tensor("xth", [D, T], F32R, kind="ExternalInput")       # x[b].T hi
    xtl = nc.dram_tensor("xtl", [D, T], F32R, kind="ExternalInput")       # x[b].T lo
    wqkh = nc.dram_tensor("wqkh", [D, 2 * HG * P], F32R, kind="ExternalInput")
    wqkl = nc.dram_tensor("wqkl", [D, 2 * HG * P], F32R, kind="ExternalInput")
    wvh = nc.dram_tensor("wvh", [D, HG * P], F32R, kind="ExternalInput")
    wvl = nc.dram_tensor("wvl", [D, HG * P], F32R, kind="ExternalInput")
    wo = nc.dram_tensor("wo", [HG * P, D], F32, kind="ExternalInput")
    t128 = nc.dram_tensor("t128", [P, T], F32, kind="ExternalInput")
    qscale = nc.dram_tensor("qscale", [P, NT], F32, kind="ExternalInput")
    trimask = nc.dram_tensor("trimask", [P, P], F32, kind="ExternalInput")

    outp = nc.dram_tensor("outp", [T, D], F32, kind="ExternalOutput")
    khat = nc.dram_tensor("khat", [HG, P, T], F32, kind="ExternalOutput")
    vout = nc.dram_tensor("vout", [T, HG * P], F32, kind="ExternalOutput")

    with TileContext(nc) as tc:
        with (
            tc.tile_pool(name="persist", bufs=1) as pp,
            tc.tile_pool(name="const", bufs=1) as cp,
        ):
            # Persistent SBUF tensors
            qt_all = pp.tile([P, HG * T], F32, tag="qt")       # Q-tilde, [d, T] per head
            kh_all = pp.tile([P, HG * T], F32, tag="kh")       # K-hat,  [d, T] per head
            vr_all = pp.tile([P, NT * HG * P], F16, tag="vr")  # V fp16, [T-tile, hg*128]
            ot_all = pp.tile([P, HG * T], F32R, tag="ot")      # O^T fp32r per head

            sc_q = cp.tile([P, NT], F32, tag="scq")
            mask_t = cp.tile([P, P], F32, tag="mask")
            nc.sync.dma_start(sc_q[:], qscale[:])
            nc.sync.dma_start(mask_t[:], trimask[:])

            # ================= Phase 1a: Q/K projection (split fp32r) =====
            with (
                tc.tile_pool(name="p1", bufs=4) as sp1,
                tc.tile_pool(name="t128p", bufs=1) as tp,
                tc.tile_pool(name="ps1", bufs=8, space="PSUM") as ps1,
            ):
                t128_t = tp.tile([P, T], F32, tag="t128")
                nc.sync.dma_start(t128_t[:], t128[:])
                for n in range(4):          # T chunks of 512
                    nsl = slice(n * 512, (n + 1) * 512)
                    psm = [ps1.tile([P, 512], F32, tag="qk_ps", name=f"qkps_{n}_{m}") for m in range(8)]
                    for kc in range(16):
                        xth_t = sp1.tile([P, 512], F32R, tag="xth1")
                        xtl_t = sp1.tile([P, 512], F32R, tag="xtl1")
                        nc.sync.dma_start(xth_t[:], xth[kc * P:(kc + 1) * P, nsl])
                        nc.sync.dma_start(xtl_t[:], xtl[kc * P:(kc + 1) * P, nsl])
                        wqkh_t = sp1.tile([P, 1024], F32R, tag="wqkh")
                        wqkl_t = sp1.tile([P, 1024], F32R, tag="wqkl")
                        nc.sync.dma_start(wqkh_t[:], wqkh[kc * P:(kc + 1) * P, :])
                        nc.sync.dma_start(wqkl_t[:], wqkl[kc * P:(kc + 1) * P, :])
                        for m in range(8):
                            msl = slice(m * P, (m + 1) * P)
                            nc.tensor.matmul(psm[m][:], wqkh_t[:, msl], xth_t[:],
                                             start=(kc == 0), stop=False)
                            nc.tensor.matmul(psm[m][:], wqkl_t[:, msl], xth_t[:],
                                             start=False, stop=False)
                            nc.tensor.matmul(psm[m][:], wqkh_t[:, msl], xtl_t[:],
                                             start=False, stop=(kc == 15))
                    for m in range(8):
                        if m < 4:
                            nc.scalar.copy(qt_all[:, m * T + n * 512: m * T + (n + 1) * 512], psm[m][:])
                        else:
                            h = m - 4
                            dst = kh_all[:, h * T + n * 512: h * T + (n + 1) * 512]
                            nc.vector.tensor_mul(dst, psm[m][:], t128_t[:, nsl])

            # ================= Phase 1b: V projection (split fp32r) =======
            with (
                tc.tile_pool(name="p1b", bufs=4) as sp2,
                tc.tile_pool(name="ps2", bufs=6, space="PSUM") as ps2,
            ):
                for n in range(4):
                    psv = [ps2.tile([P, 512], F32, tag="v_ps", name=f"vps_{n}_{j}") for j in range(4)]
                    for kc in range(16):
                        xth_t = sp2.tile([P, 512], F32R, tag="xth2")
                        xtl_t = sp2.tile([P, 512], F32R, tag="xtl2")
                        nc.sync.dma_start(xth_t[:], xth[kc * P:(kc + 1) * P, n * 512:(n + 1) * 512])
                        nc.sync.dma_start(xtl_t[:], xtl[kc * P:(kc + 1) * P, n * 512:(n + 1) * 512])
                        wvh_t = sp2.tile([P, 512], F32R, tag="wvh")
                        wvl_t = sp2.tile([P, 512], F32R, tag="wvl")
                        nc.sync.dma_start(wvh_t[:], wvh[kc * P:(kc + 1) * P, :])
                        nc.sync.dma_start(wvl_t[:], wvl[kc * P:(kc + 1) * P, :])
                        for j in range(4):
                            jsl = slice(j * P, (j + 1) * P)
                            nc.tensor.matmul(psv[j][:], xth_t[:, jsl], wvh_t[:],
                                             start=(kc == 0), stop=False)
                            nc.tensor.matmul(psv[j][:], xtl_t[:, jsl], wvh_t[:],
                                             start=False, stop=False)
                            nc.tensor.matmul(psv[j][:], xth_t[:, jsl], wvl_t[:],
                                             start=False, stop=(kc == 15))
                    for j in range(4):
                        tt = n * 4 + j
                        vst = sp2.tile([P, 512], F32, tag="vstage")
                        nc.scalar.copy(vst[:], psv[j][:])
                        nc.sync.dma_start(vout[tt * P:(tt + 1) * P, :], vst[:])
                        nc.vector.tensor_copy(vr_all[:, tt * 512:(tt + 1) * 512], vst[:])

            wo_ts = []
            for h in range(HG):
                wo_t = pp.tile([P, D], F32R, tag=f"wo{h}", name=f"wo_{h}")
                nc.gpsimd.dma_start(wo_t[:], wo[h * P:(h + 1) * P, :])
                wo_ts.append(wo_t)

            for h in range(HG):
                nc.gpsimd.dma_start(khat[h], kh_all[:, h * T:(h + 1) * T])

            # ========== Phase 2: attention + fused output projection ======
            with (
                tc.tile_pool(name="p2", bufs=3) as ap,
                tc.tile_pool(name="p2s", bufs=4) as aps,
                tc.tile_pool(name="ptp", bufs=2) as ptp,
                tc.tile_pool(name="ps_s", bufs=6, space="PSUM") as psA,
                tc.tile_pool(name="ps_o", bufs=2, space="PSUM") as psO,
            ):
                def softmax_qtile(h, i, ptile):
                    hq = qt_all[:, h * T:(h + 1) * T]
                    hk = kh_all[:, h * T:(h + 1) * T]
                    W = P * (i + 1)
                    widths = [512] * (W // 512) + ([W % 512] if W % 512 else [])
                    nch = len(widths)
                    mx = aps.tile([P, 4], F32, tag="mx", name=f"mx_{h}_{i}")
                    dn = aps.tile([P, 4], F32, tag="dn", name=f"dn_{h}_{i}")
                    chunks = []
                    c0 = 0
                    for ci, wd in enumerate(widths):
                        s_ps = psA.tile([P, 512], F32, tag="ps", name=f"s_{h}_{i}_{ci}")
                        nc.tensor.matmul(
                            s_ps[:, :wd], hq[:, i * P:(i + 1) * P], hk[:, c0:c0 + wd],
                            start=True, stop=True)
                        if ci == nch - 1:
                            doff = wd - P
                            nc.vector.tensor_add(
                                s_ps[:, doff:doff + P], s_ps[:, doff:doff + P], mask_t[:])
                        nc.vector.reduce_max(
                            mx[:, ci:ci + 1], s_ps[:, :wd], axis=mybir.AxisListType.X)
                        chunks.append((s_ps, c0, wd))
                        c0 += wd
                    m_f = aps.tile([P, 1], F32, tag="mf", name=f"mf_{h}_{i}")
                    bias = aps.tile([P, 1], F32, tag="bias", name=f"bias_{h}_{i}")
                    nc.vector.reduce_max(m_f[:], mx[:, :nch], axis=mybir.AxisListType.X)
                    nc.vector.tensor_scalar(
                        out=bias[:], in0=m_f[:], scalar1=sc_q[:, i:i + 1],
                        scalar2=-1.0, op0=mybir.AluOpType.mult,
                        op1=mybir.AluOpType.mult)
                    for ci, (s_ps, c0, wd) in enumerate(chunks):
                        nc.scalar.activation(
                            ptile[:, c0:c0 + wd], s_ps[:, :wd],
                            mybir.ActivationFunctionType.Exp,
                            bias=bias[:, 0:1], scale=sc_q[:, i:i + 1],
                            accum_out=dn[:, ci:ci + 1])
                    dsum = aps.tile([P, 1], F32, tag="dsum", name=f"ds_{h}_{i}")
                    rec = aps.tile([P, 1], F32, tag="rec", name=f"rec_{h}_{i}")
                    nc.vector.reduce_sum(dsum[:], dn[:, :nch], axis=mybir.AxisListType.X)
                    nc.vector.reciprocal(rec[:], dsum[:])
                    nc.vector.tensor_scalar_mul(ptile[:, :W], ptile[:, :W], rec[:, 0:1])

                for pr in range(NT // 2):     # q-pairs
                    i0, i1 = 2 * pr, 2 * pr + 1

                    def pv_block(h, big_pt):
                        ot_ps = psO.tile([P, 256], F32, tag="ot", name=f"ot_{h}_{pr}")
                        for kt in range(i1 + 1):
                            nc.tensor.matmul(
                                ot_ps[:, 0:256],
                                vr_all[:, kt * 512 + h * P: kt * 512 + (h + 1) * P],
                                big_pt[:, kt, :],
                                start=(kt == 0), stop=(kt == i1))
                        nc.vector.tensor_copy(
                            ot_all[:, h * T + i0 * P: h * T + (i1 + 1) * P], ot_ps[:, 0:256])

                    for h in range(HG):
                        pt0 = ap.tile([P, T], F16, tag="pt0", name=f"p0_{h}_{pr}")
                        pt1 = ap.tile([P, T], F16, tag="pt1", name=f"p1_{h}_{pr}")
                        softmax_qtile(h, i0, pt0)
                        softmax_qtile(h, i1, pt1)
                        big_pt = ptp.tile([P, NT, 256], F16, tag="bigpt", name=f"bpt_{h}_{pr}")
                        nc.sync.dma_start_transpose(
                            big_pt[:, 0:i1, 0:P], pt0[:, 0:i1 * P])
                        nc.gpsimd.memset(big_pt[:, i1, 0:P], 0.0)
                        nc.sync.dma_start_transpose(
                            big_pt[:, 0:i1 + 1, P:256], pt1[:, 0:(i1 + 1) * P])
                        pv_block(h, big_pt)

            # -------- output projection (separate tail phase) --------
            with (
                tc.tile_pool(name="p3", bufs=3) as op,
                tc.tile_pool(name="ps_w", bufs=4, space="PSUM") as psW,
            ):
                for pr in range(NT // 2):
                    for half, tt in enumerate((2 * pr, 2 * pr + 1)):
                        for n in range(4):
                            po = psW.tile([P, 512], F32, tag="po", name=f"po_{pr}_{half}_{n}")
                            for h in range(HG):
                                nc.tensor.matmul(
                                    po[:],
                                    ot_all[:, h * T + tt * P: h * T + (tt + 1) * P],
                                    wo_ts[h][:, n * 512:(n + 1) * 512],
                                    start=(h == 0), stop=(h == HG - 1))
                            ost = op.tile([P, 512], F32, tag="ostage", name=f"os_{pr}_{half}_{n}")
                            nc.scalar.copy(ost[:], po[:])
                            nc.sync.dma_start(outp[tt * P:(tt + 1) * P, n * 512:(n + 1) * 512], ost[:])

    nc.finalize()
    _NC_CACHE["nc"] = nc
    return nc


def _host_prep(x, Wqkv, Wout):
    """Build per-core input maps. RoPE folded into Wq/Wk (exact: cos/sin
    are linear in t for this reference)."""
    D2 = HEAD_DIM // 2
    inv = 1.0 / (10000.0 ** (np.arange(0, HEAD_DIM, 2, dtype=np.float64) / HEAD_DIM))
    emb1 = np.concatenate([inv, inv])           # freqs row at t=1
    alpha = emb1[::2].copy()                    # [64]
    beta = emb1[1::2].copy()                    # [64]

    Wq64 = np.asarray(Wqkv[:, 0:D], np.float64)
    Wk64 = np.asarray(Wqkv[:, D:2 * D], np.float64)

    tvals = np.arange(T, dtype=np.float64)
    t128v = np.broadcast_to(tvals[None, :].astype(np.float32), (P, T)).copy()
    qs = np.empty((P, NT), np.float32)
    for i in range(NT):
        tq = np.maximum(np.arange(i * P, (i + 1) * P, dtype=np.float64), 1e-3)
        qs[:, i] = (tq / math.sqrt(HEAD_DIM)).astype(np.float32)
    tri = np.where(np.arange(P)[:, None] >= np.arange(P)[None, :], 0.0, -1e30).astype(np.float32)

    def split12(a):
        a = np.ascontiguousarray(a, np.float32)
        bits = a.view(np.uint32)
        hi = ((bits + 0x800) & 0xFFFFF000).view(np.float32).copy()
        hi[~np.isfinite(hi)] = a[~np.isfinite(hi)]
        lo = (a - hi).astype(np.float32)
        return hi, lo

    def fold(Wh):  # Wh [D, 128] fp64 -> folded [D, 128] (d-tilde order)
        We, Wo_ = Wh[:, 0::2], Wh[:, 1::2]      # [D, 64] each
        lo = We * alpha[None, :] - Wo_ * beta[None, :]
        hi = We * beta[None, :] + Wo_ * alpha[None, :]
        return np.concatenate([lo, hi], axis=1)

    perm = np.empty(HEAD_DIM, np.int64)
    perm[:D2] = np.arange(0, HEAD_DIM, 2)       # d-tilde j -> original dim
    perm[D2:] = np.arange(1, HEAD_DIM, 2)

    in_maps = []
    for c in range(N_CORES):
        b, hg = divmod(c, HG)
        heads = range(hg * HG, hg * HG + HG)
        wq_f = np.concatenate([fold(Wq64[:, gh * P:(gh + 1) * P]) for gh in heads], axis=1)
        wk_f = np.concatenate([fold(Wk64[:, gh * P:(gh + 1) * P]) for gh in heads], axis=1)
        wqk = np.concatenate([wq_f, wk_f], axis=1).astype(np.float32)
        wqkh, wqkl = split12(wqk)
        wv = np.ascontiguousarray(
            Wqkv[:, 2 * D + hg * HG * P: 2 * D + (hg + 1) * HG * P]).astype(np.float32)
        wvh, wvl = split12(wv)
        wo_rows = np.ascontiguousarray(
            np.asarray(Wout)[hg * HG * P:(hg + 1) * HG * P, :]).astype(np.float32)
        xt = np.ascontiguousarray(np.asarray(x)[b].T).astype(np.float32)
        xth, xtl = split12(xt)
        in_maps.append({
            "xth": xth, "xtl": xtl, "wqkh": wqkh, "wqkl": wqkl,
            "wvh": wvh, "wvl": wvl, "wo": wo_rows,
            "t128": t128v, "qscale": qs, "trimask": tri,
        })
    return in_maps, perm


def kernel(x, attn_mask, Wqkv, Wout, _trace=False):
    x = np.asarray(x); Wqkv = np.asarray(Wqkv); Wout = np.asarray(Wout)
    in_maps, perm = _host_prep(x, Wqkv, Wout)
    nc = build_nc()
    res = run_bass_kernel_spmd(nc, in_maps, core_ids=list(range(N_CORES)),
                               trace=_trace)

    out = np.zeros((B, T, D), np.float32)
    k = np.empty((B, N_HEAD, T, HEAD_DIM), np.float32)
    v = np.empty((B, N_HEAD, T, HEAD_DIM), np.float32)
    for c in range(N_CORES):
        b, hg = divmod(c, HG)
        r = res.results[c]
        out[b] += r["outp"]
        for h in range(HG):
            gh = hg * HG + h
            tmp = np.empty((HEAD_DIM, T), np.float32)
            tmp[perm] = r["khat"][h]
            k[b, gh] = tmp.T
            v[b, gh] = r["vout"][:, h * P:(h + 1) * P]
    if _trace:
        kernel.last_exec_time_ns = res.exec_time_ns
    return (out, k, v)


if __name__ == "__main__":
    rng = np.random.default_rng(0)
    x = rng.standard_normal((B, T, D)).astype(np.float32)
    Wqkv = (rng.standard_normal((D, 3 * D)) / math.sqrt(D)).astype(np.float32)
    Wout = (rng.standard_normal((D, D)) / math.sqrt(D)).astype(np.float32)
    mask = np.tril(np.ones((T, T), np.int32))[None, None]
    o, kk, vv = kernel(x, mask, Wqkv, Wout)
    print("ran ok", o.shape, kk.shape, vv.shape)
